# revision 20
# baseline (speedup 1.0000x reference)
"""Causal multi-head attention (B=256, T=197, C=768, H=12, D=64) on 8 trn2 cores.

Strategy (final):
- Data-parallel over batch: 32 batches per core, no collectives.
- Host pre-transposes x to [C, T] layout per batch (xT), so Q^T/K^T come out of
  the projection matmuls directly in [C, T] layout and V in [T, C] layout.
- Per (batch, head): S^T = K^T_slice.T @ Q^T ([k, q] layout, f32 PSUM), with
  the k/q block1 triangle packed into the same PSUM bank so ONE ACT exp per
  head covers both blocks. Causal mask = multiplicative 0/1 bf16 mask (DVE)
  on the triangle regions only.
- ctx in [q, d] layout: ctx = P^T.T @ [V | 1], with P^T tiles as lhsT and the
  per-batch V tile (plus an appended ones-column) as rhs. The ones-column
  makes column 64 of each head's ctx tile the softmax denominator, i.e. a
  PER-PARTITION scalar: reciprocals run as tiny [128, 2, 1] DVE ops and
  normalization is a stride-0-broadcast DVE multiply. (v1 normalized in
  [d, q] layout, which needed a DRAM-bounce partition broadcast plus
  single-lane reciprocals = ~1.1ms of DVE/DMA critical path.)
- The normalized ctx [q, d] is transposed back to [j, t] via PE transpose-mode
  matmuls, emitted inside the head loop right when each j-block pair's data
  is ready so they sit between dense matmuls.
- Output projection is group-packed: the 4 batches' t-dims concatenate to 788
  rows = 7 M-tiles instead of 8 per group.
- SOFTWARE PIPELINE for the HAM clock-gate: the PE runs at 1.2 GHz unless
  kept busy (~3.4us activity windows). Each group's attention phase is
  interleaved ("fed") with the NEXT group's dense projection matmuls and the
  PREVIOUS group's output-projection chunks, so PE duty stays high and the
  clock stays at 2.4 GHz (~96% warm vs ~35% for the phase-separated order).
- All matmuls bf16; accumulation f32 in PSUM; softmax internals f32.

Measured (NTFF, core 0): 659us vs 1849us for the v1 baseline (2.8x).
"""

import numpy as np

B, T, C, H = 256, 197, 768, 12
D = C // H          # 64
P = 128             # partition size
CB = C // P         # 6 c-blocks
NCORES = 8
NB = B // NCORES    # 32 batches per core
G = 4               # batches per projection group
TG = G * T          # 788 tokens per group
T0 = P              # first t/k block rows (128)
T1 = T - P          # second block rows (69)
SB1 = 200           # col offset of the k/q block1 triangle inside an S tile
TPW = 200           # padded per-j-block stride in the transpose tiles (4B align)
SW = 272            # S tile width (block0 197 | pad | block1 69 | pad)
CW = 130            # per-head width inside a ctx PSUM tile (65 q0 + 65 q1)
HC = 2              # heads per ctx PSUM tile chunk
NCH = H // HC       # 4 chunks per batch

_CACHE = {}


def _split_ctrl_waits(nc):
    """The walrus encodes at most 1 sem wait per instruction (2 for
    EventSemaphore), but Tile emits instructions with several. Split excess
    waits onto NoOps inserted before the offending instruction on the same
    engine (a NoOp itself carries 1 wait)."""
    import concourse.mybir as mybir

    for fn in nc.m.functions:
        for bb in fn.blocks:
            insts = bb.instructions
            newlist = []
            changed = False
            for inst in insts:
                cap = 2 if isinstance(inst, mybir.InstEventSemaphore) else 1
                si = inst.sync_info
                waits = list(si.on_wait) if si and si.on_wait else []
                if len(waits) > cap:
                    changed = True
                    head, rest = waits[:-cap], waits[-cap:]
                    for w in head:
                        nop = mybir.InstNoOp(
                            name=nc.get_next_instruction_name(),
                            bass_nofuse=True,
                            engine=inst.engine,
                            sync_info=mybir.SyncInfo(on_wait=[w], on_update=[]),
                        )
                        newlist.append(nop)
                    inst.sync_info = mybir.SyncInfo(
                        on_wait=rest,
                        on_update=list(si.on_update) if si.on_update else [],
                    )
                newlist.append(inst)
            if changed:
                bb.instructions = newlist


def _dedup_ldweights(nc):
    """Delete an InstLdweights that reloads the exact weights already loaded
    by the previous PE ldweights with no different load in between (our
    512/276-column chunk pairs share lhsT). Only drops wait-free duplicates."""
    import concourse.mybir as mybir

    ndrop = 0
    for fn in nc.m.functions:
        for bb in fn.blocks:
            insts = bb.instructions
            newlist = []
            last_sig = None
            changed = False
            for inst in insts:
                if inst.engine != mybir.EngineType.PE:
                    newlist.append(inst)
                    continue
                if type(inst).__name__ == "InstLdweights":
                    si = inst.sync_info
                    nw = len(si.on_wait) if si and si.on_wait else 0
                    nu = len(si.on_update) if si and si.on_update else 0
                    sig = (str(inst.ins[0]), str(inst.tile_position),
                           str(inst.tile_size), str(inst.is_transpose),
                           str(inst.perf_mode))
                    if sig == last_sig and nw == 0 and nu == 0:
                        changed = True
                        ndrop += 1
                        continue  # drop duplicate
                    last_sig = sig
                newlist.append(inst)
            if changed:
                bb.instructions = newlist
    return ndrop


def build_nc(nb=NB, split_waits=True, repeat=1):
    import concourse.bass as bass
    import concourse.mybir as mybir
    from concourse.tile import TileContext

    f32 = mybir.dt.float32
    bf16 = mybir.dt.bfloat16
    Exp = mybir.ActivationFunctionType.Exp
    Copy = mybir.ActivationFunctionType.Copy

    ng = nb // G

    nc = bass.Bass()
    xT = nc.declare_dram_parameter("xT", [CB, P, nb * T], bf16, isOutput=False)
    wq = nc.declare_dram_parameter("wq", [CB, P, C], bf16, isOutput=False)
    wk = nc.declare_dram_parameter("wk", [CB, P, C], bf16, isOutput=False)
    wv = nc.declare_dram_parameter("wv", [CB, P, C], bf16, isOutput=False)
    wo = nc.declare_dram_parameter("wo", [CB, P, C], bf16, isOutput=False)
    maskp = nc.declare_dram_parameter("mask", [P, T], bf16, isOutput=False)
    identp = nc.declare_dram_parameter("ident", [P, P], bf16, isOutput=False)
    out = nc.declare_dram_parameter("out", [nb, T, C], f32, isOutput=True)

    with TileContext(nc) as tc:
        with (
            tc.tile_pool(name="weights", bufs=1) as wpool,
            tc.tile_pool(name="x", bufs=3) as xpool,
            tc.tile_pool(name="qk", bufs=2) as qkpool,
            tc.tile_pool(name="v", bufs=18) as vpool,
            tc.tile_pool(name="p", bufs=4) as ppool,
            tc.tile_pool(name="inv", bufs=4) as invpool,
            tc.tile_pool(name="cq", bufs=2) as cqpool,
            tc.tile_pool(name="ctxT", bufs=6) as ctxTpool,
            tc.tile_pool(name="outsb", bufs=3) as outsbpool,
            tc.tile_pool(name="bigps", bufs=2, space="PSUM") as bigps,
            tc.tile_pool(name="sps", bufs=2, space="PSUM") as sps,
            tc.tile_pool(name="cxps", bufs=2, space="PSUM") as cxps,
        ):
            # --- static tiles ---
            wq_sb = wpool.tile([P, CB, C], bf16, tag="wq")
            wk_sb = wpool.tile([P, CB, C], bf16, tag="wk")
            wv_sb = wpool.tile([P, CB, C], bf16, tag="wv")
            wo_sb = wpool.tile([P, CB, C], bf16, tag="wo")
            mask_sb = wpool.tile([P, T], bf16, tag="mask")
            ident_sb = wpool.tile([P, P], bf16, tag="ident")
            st = {}
            seq = list(range(ng)) * repeat
            nseq = len(seq)

            def emit_xT(i, split=False):
                gd = seq[i]
                xT_sb = xpool.tile([P, CB, TG], bf16, tag="xT",
                                   name=f"xT_{i}")
                if split:
                    # per-ib DMAs so the first projection matmul (which
                    # consumes ib-chunks in order) can start ~5us earlier
                    for ib in range(CB):
                        nc.sync.dma_start(
                            out=xT_sb[:, ib, :],
                            in_=xT[ib, :, gd * TG:(gd + 1) * TG])
                else:
                    nc.sync.dma_start(
                        out=xT_sb[:],
                        in_=xT[:, :, gd * TG:(gd + 1) * TG].rearrange(
                            "ib p t -> p ib t"))
                st[("x", i)] = xT_sb

            def emit_qk_chunk(i, idx):
                # idx 0..11: even -> q, odd -> k; cb = idx // 2
                if idx == 0:
                    st[("qT", i)] = qkpool.tile([P, CB, TG], bf16, tag="qT",
                                                name=f"qT_{i}")
                    st[("kT", i)] = qkpool.tile([P, CB, TG], bf16, tag="kT",
                                                name=f"kT_{i}")
                w_sb, dst = ((wq_sb, st[("qT", i)]) if idx % 2 == 0
                             else (wk_sb, st[("kT", i)]))
                st[("qkn", i)] = idx + 1
                cb = idx // 2
                xT_sb = st[("x", i)]
                ps = bigps.tile([P, TG], f32, tag="ps", name=f"qkps_{i}_{idx}")
                for ib in range(CB):
                    lhs = w_sb[:, ib, cb * P:(cb + 1) * P]
                    nc.tensor.matmul(
                        ps[:, 0:512], lhsT=lhs, rhs=xT_sb[:, ib, 0:512],
                        start=(ib == 0), stop=(ib == CB - 1))
                    nc.tensor.matmul(
                        ps[:, 512:TG], lhsT=lhs, rhs=xT_sb[:, ib, 512:TG],
                        start=(ib == 0), stop=(ib == CB - 1))
                nc.scalar.activation(dst[:, cb, :], ps[:, :], Copy)

            def emit_v_chunk(i, idx):
                # idx 0..7: b = idx // 2, tb = idx % 2
                b, tb = idx // 2, idx % 2
                rows = T0 if tb == 0 else T1
                col0 = b * T + tb * P
                xT_sb = st[("x", i)]
                ps = bigps.tile([P, TG], f32, tag="ps", name=f"vps_{i}_{idx}")
                for ib in range(CB):
                    lhs = xT_sb[:, ib, col0:col0 + rows]
                    nc.tensor.matmul(
                        ps[0:rows, 0:512], lhsT=lhs, rhs=wv_sb[:, ib, 0:512],
                        start=(ib == 0), stop=(ib == CB - 1))
                    nc.tensor.matmul(
                        ps[0:rows, 512:C], lhsT=lhs, rhs=wv_sb[:, ib, 512:C],
                        start=(ib == 0), stop=(ib == CB - 1))
                v_sb = vpool.tile([P, H, D + 1], bf16, tag="v",
                                  name=f"v_{i}_{idx}")
                nc.vector.tensor_copy(
                    out=v_sb[0:rows, :, 0:D],
                    in_=ps[0:rows, 0:C].rearrange("p (h d) -> p h d", d=D))
                nc.vector.memset(v_sb[0:rows, :, D], 1.0)
                st[("v", i, b, tb)] = v_sb

            NM = (TG + P - 1) // P  # out-projection M-tiles per group (7)

            def emit_out_chunk(i, m):
                """One M-tile of the group-packed output projection: the four
                batches' t-dims concatenate to 788 rows = 7 tiles of <=128
                (vs 8 when tiled per batch: 197 -> 128+69 wastes M)."""
                gd = seq[i]
                rows = min(P, TG - m * P)
                ps = bigps.tile([P, TG], f32, tag="ps", name=f"ops_{i}_{m}")
                for jb in range(CB):
                    ct = st[("ctxT", i, jb // 2)]
                    lhs = ct[:, jb % 2, m * P:m * P + rows]
                    nc.tensor.matmul(
                        ps[0:rows, 0:512], lhsT=lhs,
                        rhs=wo_sb[:, jb, 0:512],
                        start=(jb == 0), stop=(jb == CB - 1))
                    nc.tensor.matmul(
                        ps[0:rows, 512:C], lhsT=lhs,
                        rhs=wo_sb[:, jb, 512:C],
                        start=(jb == 0), stop=(jb == CB - 1))
                out_sb = outsbpool.tile([P, C], f32, tag="out",
                                        name=f"osb_{i}_{m}")
                nc.scalar.activation(out_sb[0:rows, :], ps[0:rows, 0:C],
                                     Copy)
                nc.sync.dma_start(
                    out=out.rearrange("b t c -> (b t) c")[
                        gd * TG + m * P:gd * TG + m * P + rows, :],
                    in_=out_sb[0:rows, :])
                if m == NM - 1:
                    # group fully projected: release the ctxT references so
                    # the epilogue doesn't re-emit (and so tile lifetimes end)
                    for jp in range(CB // 2):
                        st.pop(("ctxT", i, jp), None)

            def emit_out_group(i):
                for m in range(NM):
                    emit_out_chunk(i, m)

            def proj_feeder(i):
                """Thunks interleaved into group i-1's attention so the PE
                duty cycle stays high (keeps the HAM clock-gate at 8/8):
                group i-2's output projection, then group i's projections.
                proj_feeder(1) first yields the tail of group 0's own
                projections (the prologue emits only what batch 0 needs)."""
                if i == 1:
                    for idx in range(2, 12):
                        yield lambda idx=idx: emit_qk_chunk(0, idx)
                    for idx in range(2, 8):
                        yield lambda idx=idx: emit_v_chunk(0, idx)
                if i - 2 >= 0:
                    for m in range(NM):
                        yield lambda m=m: emit_out_chunk(i - 2, m)
                if 1 <= i < nseq:
                    for idx in range(12):
                        yield lambda idx=idx: emit_qk_chunk(i, idx)
                    for idx in range(8):
                        yield lambda idx=idx: emit_v_chunk(i, idx)

            def feed(feeder):
                thunk = next(feeder, None)
                if thunk is not None:
                    thunk()
                    return True
                return False

            def attn_batch(i, b, feeder):
                gd = seq[i]
                # force-drain the feeder until this batch's inputs exist
                while (("v", i, b, 0) not in st or ("v", i, b, 1) not in st
                       or ("qT", i) not in st):
                    if not feed(feeder):
                        break
                qT_sb, kT_sb = st[("qT", i)], st[("kT", i)]
                vb0 = st[("v", i, b, 0)]
                vb1 = st[("v", i, b, 1)]
                ctxq0 = cqpool.tile([P, C], bf16, tag="cq0",
                                    name=f"cq0_{i}_{b}")
                ctxq1 = cqpool.tile([T1, C], bf16, tag="cq1",
                                    name=f"cq1_{i}_{b}")
                ptiles = {}
                chunk = {}

                def s_head(h):
                    j, hi = h // 2, h % 2
                    base = hi * D
                    qh = qT_sb[base:base + D, j, b * T:(b + 1) * T]
                    kh = kT_sb[base:base + D, j, b * T:(b + 1) * T]
                    s = sps.tile([P, SW], f32, tag="s", name=f"s_{i}_{b}_{h}")
                    nc.tensor.matmul(s[:, 0:T], lhsT=kh[:, 0:P], rhs=qh,
                                     start=True, stop=True,
                                     skip_group_check=True)
                    nc.tensor.matmul(s[0:T1, SB1:SB1 + T1],
                                     lhsT=kh[:, P:T], rhs=qh[:, P:T],
                                     start=True, stop=True,
                                     skip_group_check=True)
                    p = ppool.tile([P, SW], bf16, tag="p",
                                   name=f"p_{i}_{b}_{h}")
                    nc.scalar.activation(p[:], s[:], Exp)
                    # causal mask: multiplicative 0/1 on the triangles
                    nc.vector.tensor_mul(p[:, 0:P], p[:, 0:P],
                                         mask_sb[:, 0:P])
                    nc.vector.tensor_mul(p[0:T1, SB1:SB1 + T1],
                                         p[0:T1, SB1:SB1 + T1],
                                         mask_sb[0:T1, 0:T1])
                    ptiles[h] = p

                def ctx_head(h):
                    r = h % HC
                    if r == 0:
                        chunk["ctx"] = cxps.tile([P, HC * CW], f32, tag="cx",
                                                 name=f"cx_{i}_{b}_{h}")
                        chunk["inv"] = invpool.tile([P, HC, 2], f32,
                                                    tag="inv",
                                                    name=f"inv_{i}_{b}_{h}")
                    ctx_ps, inv = chunk["ctx"], chunk["inv"]
                    p = ptiles.pop(h)
                    col = r * CW
                    # q-block0: all needed k are in k-block0
                    nc.tensor.matmul(ctx_ps[:, col:col + D + 1],
                                     lhsT=p[:, 0:P], rhs=vb0[:, h, :],
                                     start=True, stop=True,
                                     skip_group_check=True)
                    # q-block1: full k-block0 + the block1 triangle
                    nc.tensor.matmul(ctx_ps[0:T1, col + D + 1:col + CW],
                                     lhsT=p[:, P:T], rhs=vb0[:, h, :],
                                     start=True, stop=False,
                                     skip_group_check=True)
                    nc.tensor.matmul(ctx_ps[0:T1, col + D + 1:col + CW],
                                     lhsT=p[0:T1, SB1:SB1 + T1],
                                     rhs=vb1[0:T1, h, :],
                                     start=False, stop=True,
                                     skip_group_check=True)
                    if r == HC - 1:
                        h0 = h - (HC - 1)
                        ctx_r = ctx_ps.rearrange("p (h c) -> p h c", c=CW)
                        # denominators sit at columns D / CW-1 per head
                        nc.vector.reciprocal(
                            inv[:, :, 0:1], ctx_r[:, :, D:D + 1])
                        nc.vector.reciprocal(
                            inv[0:T1, :, 1:2], ctx_r[0:T1, :, CW - 1:CW])
                        # normalize + evict to ctxq tiles (bf16)
                        nc.vector.tensor_mul(
                            ctxq0[:, h0 * D:(h0 + HC) * D].rearrange(
                                "p (h d) -> p h d", d=D),
                            ctx_r[:, :, 0:D],
                            inv[:, :, 0:1].to_broadcast((P, HC, D)))
                        nc.vector.tensor_mul(
                            ctxq1[:, h0 * D:(h0 + HC) * D].rearrange(
                                "p (h d) -> p h d", d=D),
                            ctx_r[0:T1, :, D + 1:CW - 1],
                            inv[0:T1, :, 1:2].to_broadcast((T1, HC, D)))

                def transpose_pair(jp):
                    # ctx^T[j,t] for j-blocks 2jp/2jp+1 (heads 4jp..4jp+3);
                    # sits between dense matmuls so the HAM clock-gate stays
                    # open (transpose-mode MMs don't count as PE activity)
                    tps = cxps.tile([P, 2 * TPW], bf16, tag="cx",
                                    name=f"tps_{i}_{b}_{jp}")
                    for u in range(2):
                        jb = 2 * jp + u
                        coff = u * TPW
                        nc.tensor.matmul(
                            tps[:, coff:coff + P],
                            lhsT=ctxq0[:, jb * P:(jb + 1) * P],
                            rhs=ident_sb[:], is_transpose=True,
                            start=True, stop=True, skip_group_check=True)
                        nc.tensor.matmul(
                            tps[:, coff + P:coff + T],
                            lhsT=ctxq1[:, jb * P:(jb + 1) * P],
                            rhs=ident_sb[0:T1, 0:T1], is_transpose=True,
                            start=True, stop=True, skip_group_check=True)
                    if b == 0:
                        st[("ctxT", i, jp)] = ctxTpool.tile(
                            [P, 2, TG], bf16, tag="ctxT",
                            name=f"ctxT_{i}_{jp}")
                    ct = st[("ctxT", i, jp)]
                    nc.vector.tensor_copy(
                        out=ct[:, :, b * T:(b + 1) * T],
                        in_=tps.rearrange("p (u t) -> p u t", t=TPW)[:, :, 0:T])

                # interleave: S(h) then ctx(h-1); feed a projection chunk of
                # the NEXT group after every other head to keep the PE dense;
                # transpose each j-block pair as soon as its 4 heads are done
                def need_qk(h):
                    # S(h) reads q/k c-block h//2: ensure those projection
                    # chunks are EMITTED before s_head(h) (group 0 self-feeds)
                    while st.get(("qkn", i), 0) < 2 * (h // 2) + 2:
                        if not feed(feeder):
                            break

                need_qk(0)
                s_head(0)
                for h in range(1, H):
                    need_qk(h)
                    s_head(h)
                    ctx_head(h - 1)
                    feed(feeder)
                    if h in (5, 9):
                        transpose_pair((h - 5) // 4)
                ctx_head(H - 1)
                transpose_pair(2)


            # --- schedule ---
            # weight DMAs ordered so the first projections' deps land first;
            # wq/xT(0) split per-ib since the first matmuls consume them in
            # ib order (walrus subtile deps let compute start after chunk 0)
            for ib in range(CB):
                nc.sync.dma_start(out=wq_sb[:, ib, :], in_=wq[ib])
            nc.sync.dma_start(out=mask_sb[:], in_=maskp[:])
            nc.sync.dma_start(out=ident_sb[:], in_=identp[:])
            emit_xT(0, split=True)
            for ib in range(CB):
                nc.sync.dma_start(out=wk_sb[:, ib, :], in_=wk[ib])
            for ib in range(CB):
                nc.sync.dma_start(out=wv_sb[:, ib, :], in_=wv[ib])
            for ib in range(CB):
                nc.sync.dma_start(out=wo_sb[:, ib, :], in_=wo[ib])
            if nseq > 1:
                emit_xT(1)
            # minimal prologue: q/k c-block 0 and batch 0's v tiles — just
            # what attention(0, b=0) needs to start; the rest is fed inline
            emit_qk_chunk(0, 0)
            emit_qk_chunk(0, 1)
            emit_v_chunk(0, 0)
            emit_v_chunk(0, 1)
            for i in range(nseq):
                if i + 2 < nseq:
                    emit_xT(i + 2)
                feeder = proj_feeder(i + 1)
                for b in range(G):
                    attn_batch(i, b, feeder)
                for thunk in feeder:  # drain any unfed chunks
                    thunk()
                # drop references for this pipeline position
                for key in [("x", i), ("qT", i), ("kT", i)] + [
                        ("v", i, b, tb) for b in range(G) for tb in range(2)]:
                    st.pop(key, None)
            # epilogue: output projections not yet drained by feeders
            for i in range(nseq):
                if ("ctxT", i, 0) in st:
                    emit_out_group(i)

    _dedup_ldweights(nc)
    if split_waits:
        _split_ctrl_waits(nc)
    return nc


def _prep_core_inputs(hidden_states, Wq, Wk, Wv, Wo):
    """Host-side layout prep. Returns per-core in_maps (list of dicts)."""
    import ml_dtypes

    bf16 = ml_dtypes.bfloat16
    scale = 1.0 / np.sqrt(D)
    # xT[ib, p, b*T+t] = x[b, t, ib*128+p]
    x = np.ascontiguousarray(hidden_states.astype(np.float32))
    wq_h = np.ascontiguousarray((Wq * scale).reshape(CB, P, C).astype(bf16))
    wk_h = np.ascontiguousarray(Wk.reshape(CB, P, C).astype(bf16))
    wv_h = np.ascontiguousarray(Wv.reshape(CB, P, C).astype(bf16))
    wo_h = np.ascontiguousarray(Wo.reshape(CB, P, C).astype(bf16))
    mask = (np.arange(T)[None, :] >= np.arange(P)[:, None]).astype(bf16)
    ident = np.eye(P, dtype=bf16)

    in_maps = []
    for c in range(NCORES):
        xs = x[c * NB:(c + 1) * NB]  # [NB, T, C]
        xT = xs.reshape(NB, T, CB, P).transpose(2, 3, 0, 1).reshape(CB, P, NB * T)
        in_maps.append({
            "xT": np.ascontiguousarray(xT.astype(bf16)),
            "wq": wq_h, "wk": wk_h, "wv": wv_h, "wo": wo_h,
            "mask": mask, "ident": ident,
        })
    return in_maps


def kernel(hidden_states, Wq, bq, Wk, bk, Wv, bv, Wo, bo, counter, ucb,
           **extra):
    hidden_states = np.asarray(hidden_states)
    Wq, bq = np.asarray(Wq), np.asarray(bq)
    Wk, bk = np.asarray(Wk), np.asarray(bk)
    Wv, bv = np.asarray(Wv), np.asarray(bv)
    Wo, bo = np.asarray(Wo), np.asarray(bo)

    if np.any(bq) or np.any(bk):
        # exact numpy fallback (not expected to trigger: spec fills zeros)
        return _numpy_reference(hidden_states, Wq, bq, Wk, bk, Wv, bv, Wo, bo)

    if "nc" not in _CACHE:
        _CACHE["nc"] = build_nc()
    nc = _CACHE["nc"]
    if "runner" not in _CACHE:
        _CACHE["runner"] = _make_runner(nc)
    run, out_names, out_avals = _CACHE["runner"]

    in_maps = _prep_core_inputs(hidden_states, Wq, Wk, Wv, Wo)
    out_arrs, _ = run(in_maps)
    full = np.asarray(out_arrs[out_names.index("out")])
    out = full  # [NCORES*NB, T, C] — concat over cores is exactly batch order

    # bv/bo enter the output linearly: out += bv @ Wo + bo (attention rows sum
    # to one, so the bv term is constant across positions).
    if np.any(bv) or np.any(bo):
        out = out + (bv.astype(np.float64) @ Wo.astype(np.float64)
                     + bo.astype(np.float64)).astype(np.float32)[None, None, :]
    return out.astype(np.float32)


def _make_runner(nc):
    """Cached jitted runner (mirrors bass2jax.run_bass_via_pjrt) that keeps
    inputs device-resident so repeated calls time pure device execution."""
    import jax
    import concourse.mybir as mybir
    from concourse import bass2jax
    from concourse.bass2jax import _bass_exec_p, install_neuronx_cc_hook
    from jax.sharding import Mesh, PartitionSpec
    from jax.experimental.shard_map import shard_map

    install_neuronx_cc_hook()
    n_cores = NCORES
    partition_name = (nc.partition_id_tensor.name
                      if nc.partition_id_tensor else None)
    in_names, out_names, out_avals = [], [], []
    for alloc in nc.m.functions[0].allocations:
        if not isinstance(alloc, mybir.MemoryLocationSet):
            continue
        name = alloc.memorylocations[0].name
        if alloc.kind == "ExternalInput":
            if name != partition_name:
                in_names.append(name)
        elif alloc.kind == "ExternalOutput":
            shape = tuple(alloc.tensor_shape)
            dtype = mybir.dt.np(alloc.dtype)
            out_names.append(name)
            out_avals.append(jax.core.ShapedArray(shape, dtype))
    n_params = len(in_names)
    all_names = in_names + out_names
    if partition_name is not None:
        all_names = all_names + [partition_name]

    def _body(*args):
        operands = list(args)
        if partition_name is not None:
            operands.append(bass2jax.partition_id_tensor())
        outs = _bass_exec_p.bind(
            *operands,
            out_avals=tuple(out_avals),
            in_names=tuple(all_names),
            out_names=tuple(out_names),
            lowering_input_output_aliases=(),
            sim_require_finite=False,
            sim_require_nnan=False,
            nc=nc,
        )
        return tuple(outs)

    devices = jax.devices()[:n_cores]
    mesh = Mesh(np.asarray(devices), ("core",))
    in_specs = (PartitionSpec("core"),) * (n_params + len(out_names))
    out_specs = (PartitionSpec("core"),) * len(out_names)
    sharded = jax.jit(
        shard_map(_body, mesh=mesh, in_specs=in_specs, out_specs=out_specs,
                  check_rep=False),
        keep_unused=True,
    )

    def make_repeat(repeat):
        n_outs = len(out_names)

        def _body_r(*args):
            params = list(args[:n_params])
            outbufs = list(args[n_params:])
            outs = None
            for _ in range(repeat):
                # thread the previous iteration's outputs in as the output
                # operands: forces a data dependency so XLA cannot dedupe
                # or reorder the repeated effectful calls
                outs = _body(*params, *outbufs)
                outbufs = list(outs)
            return outs
        return jax.jit(
            shard_map(_body_r, mesh=mesh, in_specs=in_specs,
                      out_specs=out_specs, check_rep=False),
            keep_unused=True,
        )

    def run(in_maps, device_inputs=None):
        if device_inputs is None:
            concat_in = [
                np.concatenate([np.asarray(in_maps[c][nm]) for c in range(n_cores)],
                               axis=0)
                for nm in in_names
            ]
            concat_zeros = [
                np.zeros((n_cores * a.shape[0], *a.shape[1:]), a.dtype)
                for a in out_avals
            ]
            device_inputs = jax.device_put(
                concat_in + concat_zeros,
                [jax.sharding.NamedSharding(mesh, PartitionSpec("core"))]
                * (n_params + len(out_names)),
            )
        out_arrs = sharded(*device_inputs)
        jax.block_until_ready(out_arrs)
        return out_arrs, device_inputs

    run.make_repeat = make_repeat
    return run, out_names, out_avals


def _numpy_reference(hidden_states, Wq, bq, Wk, bk, Wv, bv, Wo, bo):
    x = hidden_states.astype(np.float64)
    q = (x @ Wq.astype(np.float64) + bq).reshape(B, T, H, D).transpose(0, 2, 1, 3)
    k = (x @ Wk.astype(np.float64) + bk).reshape(B, T, H, D).transpose(0, 2, 1, 3)
    v = (x @ Wv.astype(np.float64) + bv).reshape(B, T, H, D).transpose(0, 2, 1, 3)
    s = np.einsum("bhqd,bhkd->bhqk", q, k) / np.sqrt(D)
    causal = np.tril(np.ones((T, T), dtype=bool))
    s = np.where(causal, s, -np.inf)
    s = s - s.max(axis=-1, keepdims=True)
    p = np.exp(s)
    p = p / p.sum(axis=-1, keepdims=True)
    ctx = np.einsum("bhqk,bhkd->bhqd", p, v).transpose(0, 2, 1, 3).reshape(B, T, C)
    return (ctx @ Wo.astype(np.float64) + bo).astype(np.float32)


# revision 21
# speedup vs baseline: 1.1949x; 1.1949x over previous
"""Causal multi-head attention (B=256, T=197, C=768, H=12, D=64) on 8 trn2 cores.

Strategy (final):
- Data-parallel over batch: 32 batches per core, no collectives.
- Host pre-transposes x to [C, T] layout per batch (xT), so Q^T/K^T come out of
  the projection matmuls directly in [C, T] layout and V in [T, C] layout.
- Per (batch, head): S^T = K^T_slice.T @ Q^T ([k, q] layout, f32 PSUM), with
  the k/q block1 triangle packed into the same PSUM bank so ONE ACT exp per
  head covers both blocks. Causal mask = multiplicative 0/1 bf16 mask (DVE)
  on the triangle regions only.
- ctx in [q, d] layout: ctx = P^T.T @ [V | 1], with P^T tiles as lhsT and the
  per-batch V tile (plus an appended ones-column) as rhs. The ones-column
  makes column 64 of each head's ctx tile the softmax denominator, i.e. a
  PER-PARTITION scalar: reciprocals run as tiny [128, 2, 1] DVE ops and
  normalization is a stride-0-broadcast DVE multiply. (v1 normalized in
  [d, q] layout, which needed a DRAM-bounce partition broadcast plus
  single-lane reciprocals = ~1.1ms of DVE/DMA critical path.)
- The normalized ctx [q, d] is transposed back to [j, t] via PE transpose-mode
  matmuls, emitted inside the head loop right when each j-block pair's data
  is ready so they sit between dense matmuls.
- Output projection is group-packed: the 4 batches' t-dims concatenate to 788
  rows = 7 M-tiles instead of 8 per group.
- SOFTWARE PIPELINE for the HAM clock-gate: the PE runs at 1.2 GHz unless
  kept busy (~3.4us activity windows). Each group's attention phase is
  interleaved ("fed") with the NEXT group's dense projection matmuls and the
  PREVIOUS group's output-projection chunks, so PE duty stays high and the
  clock stays at 2.4 GHz (~96% warm vs ~35% for the phase-separated order).
- All matmuls bf16; accumulation f32 in PSUM; softmax internals f32.

Measured (NTFF, core 0): 659us vs 1849us for the v1 baseline (2.8x).
"""

import numpy as np

B, T, C, H = 256, 197, 768, 12
D = C // H          # 64
P = 128             # partition size
CB = C // P         # 6 c-blocks
NCORES = 8
NB = B // NCORES    # 32 batches per core
G = 4               # batches per projection group
TG = G * T          # 788 tokens per group
T0 = P              # first t/k block rows (128)
T1 = T - P          # second block rows (69)
SB1 = 200           # col offset of the k/q block1 triangle inside an S tile
TPW = 200           # padded per-j-block stride in the transpose tiles (4B align)
SW = 272            # S tile width (block0 197 | pad | block1 69 | pad)
CW = 130            # per-head width inside a ctx PSUM tile (65 q0 + 65 q1)
HC = 2              # heads per ctx PSUM tile chunk
NCH = H // HC       # 4 chunks per batch

_CACHE = {}


def _split_ctrl_waits(nc):
    """The walrus encodes at most 1 sem wait per instruction (2 for
    EventSemaphore), but Tile emits instructions with several. Split excess
    waits onto NoOps inserted before the offending instruction on the same
    engine (a NoOp itself carries 1 wait)."""
    import concourse.mybir as mybir

    for fn in nc.m.functions:
        for bb in fn.blocks:
            insts = bb.instructions
            newlist = []
            changed = False
            for inst in insts:
                cap = 2 if isinstance(inst, mybir.InstEventSemaphore) else 1
                si = inst.sync_info
                waits = list(si.on_wait) if si and si.on_wait else []
                if len(waits) > cap:
                    changed = True
                    head, rest = waits[:-cap], waits[-cap:]
                    for w in head:
                        nop = mybir.InstNoOp(
                            name=nc.get_next_instruction_name(),
                            bass_nofuse=True,
                            engine=inst.engine,
                            sync_info=mybir.SyncInfo(on_wait=[w], on_update=[]),
                        )
                        newlist.append(nop)
                    inst.sync_info = mybir.SyncInfo(
                        on_wait=rest,
                        on_update=list(si.on_update) if si.on_update else [],
                    )
                newlist.append(inst)
            if changed:
                bb.instructions = newlist


def _dedup_ldweights(nc):
    """Delete an InstLdweights that reloads the exact weights already loaded
    by the previous PE ldweights with no different load in between (our
    512/276-column chunk pairs share lhsT). Only drops wait-free duplicates."""
    import concourse.mybir as mybir

    ndrop = 0
    for fn in nc.m.functions:
        for bb in fn.blocks:
            insts = bb.instructions
            newlist = []
            last_sig = None
            changed = False
            for inst in insts:
                if inst.engine != mybir.EngineType.PE:
                    newlist.append(inst)
                    continue
                if type(inst).__name__ == "InstLdweights":
                    si = inst.sync_info
                    nw = len(si.on_wait) if si and si.on_wait else 0
                    nu = len(si.on_update) if si and si.on_update else 0
                    sig = (str(inst.ins[0]), str(inst.tile_position),
                           str(inst.tile_size), str(inst.is_transpose),
                           str(inst.perf_mode))
                    if sig == last_sig and nw == 0 and nu == 0:
                        changed = True
                        ndrop += 1
                        continue  # drop duplicate
                    last_sig = sig
                newlist.append(inst)
            if changed:
                bb.instructions = newlist
    return ndrop


def build_nc(nb=NB, split_waits=True, repeat=1):
    import concourse.bass as bass
    import concourse.mybir as mybir
    from concourse.tile import TileContext

    f32 = mybir.dt.float32
    bf16 = mybir.dt.bfloat16
    Exp = mybir.ActivationFunctionType.Exp
    Copy = mybir.ActivationFunctionType.Copy

    ng = nb // G

    nc = bass.Bass()
    xT = nc.declare_dram_parameter("xT", [CB, P, nb * T], bf16, isOutput=False)
    wq = nc.declare_dram_parameter("wq", [CB, P, C], bf16, isOutput=False)
    wk = nc.declare_dram_parameter("wk", [CB, P, C], bf16, isOutput=False)
    wv = nc.declare_dram_parameter("wv", [CB, P, C], bf16, isOutput=False)
    wo = nc.declare_dram_parameter("wo", [CB, P, C], bf16, isOutput=False)
    maskp = nc.declare_dram_parameter("mask", [P, T], bf16, isOutput=False)
    identp = nc.declare_dram_parameter("ident", [P, P], bf16, isOutput=False)
    out = nc.declare_dram_parameter("out", [nb, T, C], f32, isOutput=True)

    with TileContext(nc) as tc:
        with (
            tc.tile_pool(name="weights", bufs=1) as wpool,
            tc.tile_pool(name="x", bufs=3) as xpool,
            tc.tile_pool(name="qk", bufs=2) as qkpool,
            tc.tile_pool(name="v", bufs=18) as vpool,
            tc.tile_pool(name="p", bufs=6) as ppool,
            tc.tile_pool(name="inv", bufs=6) as invpool,
            tc.tile_pool(name="cq", bufs=3) as cqpool,
            tc.tile_pool(name="ctxT", bufs=6) as ctxTpool,
            tc.tile_pool(name="outsb", bufs=4) as outsbpool,
            tc.tile_pool(name="bigps", bufs=2, space="PSUM") as bigps,
            tc.tile_pool(name="sps", bufs=2, space="PSUM") as sps,
            tc.tile_pool(name="cxps", bufs=2, space="PSUM") as cxps,
        ):
            # --- static tiles ---
            wq_sb = wpool.tile([P, CB, C], bf16, tag="wq")
            wk_sb = wpool.tile([P, CB, C], bf16, tag="wk")
            wv_sb = wpool.tile([P, CB, C], bf16, tag="wv")
            wo_sb = wpool.tile([P, CB, C], bf16, tag="wo")
            mask_sb = wpool.tile([P, T], bf16, tag="mask")
            ident_sb = wpool.tile([P, P], bf16, tag="ident")
            st = {}
            seq = list(range(ng)) * repeat
            nseq = len(seq)

            def emit_xT(i, split=False):
                gd = seq[i]
                xT_sb = xpool.tile([P, CB, TG], bf16, tag="xT",
                                   name=f"xT_{i}")
                if split:
                    # per-ib DMAs so the first projection matmul (which
                    # consumes ib-chunks in order) can start ~5us earlier
                    for ib in range(CB):
                        nc.sync.dma_start(
                            out=xT_sb[:, ib, :],
                            in_=xT[ib, :, gd * TG:(gd + 1) * TG])
                else:
                    nc.sync.dma_start(
                        out=xT_sb[:],
                        in_=xT[:, :, gd * TG:(gd + 1) * TG].rearrange(
                            "ib p t -> p ib t"))
                st[("x", i)] = xT_sb

            def emit_qk_chunk(i, idx):
                # idx 0..11: even -> q, odd -> k; cb = idx // 2
                if idx == 0:
                    st[("qT", i)] = qkpool.tile([P, CB, TG], bf16, tag="qT",
                                                name=f"qT_{i}")
                    st[("kT", i)] = qkpool.tile([P, CB, TG], bf16, tag="kT",
                                                name=f"kT_{i}")
                w_sb, dst = ((wq_sb, st[("qT", i)]) if idx % 2 == 0
                             else (wk_sb, st[("kT", i)]))
                st[("qkn", i)] = idx + 1
                cb = idx // 2
                xT_sb = st[("x", i)]
                ps = bigps.tile([P, TG], f32, tag="ps", name=f"qkps_{i}_{idx}")
                for ib in range(CB):
                    lhs = w_sb[:, ib, cb * P:(cb + 1) * P]
                    nc.tensor.matmul(
                        ps[:, 0:512], lhsT=lhs, rhs=xT_sb[:, ib, 0:512],
                        start=(ib == 0), stop=(ib == CB - 1))
                    nc.tensor.matmul(
                        ps[:, 512:TG], lhsT=lhs, rhs=xT_sb[:, ib, 512:TG],
                        start=(ib == 0), stop=(ib == CB - 1))
                nc.scalar.activation(dst[:, cb, :], ps[:, :], Copy)

            def emit_v_chunk(i, idx):
                # idx 0..7: b = idx // 2, tb = idx % 2
                b, tb = idx // 2, idx % 2
                rows = T0 if tb == 0 else T1
                col0 = b * T + tb * P
                xT_sb = st[("x", i)]
                ps = bigps.tile([P, TG], f32, tag="ps", name=f"vps_{i}_{idx}")
                for ib in range(CB):
                    lhs = xT_sb[:, ib, col0:col0 + rows]
                    nc.tensor.matmul(
                        ps[0:rows, 0:512], lhsT=lhs, rhs=wv_sb[:, ib, 0:512],
                        start=(ib == 0), stop=(ib == CB - 1))
                    nc.tensor.matmul(
                        ps[0:rows, 512:C], lhsT=lhs, rhs=wv_sb[:, ib, 512:C],
                        start=(ib == 0), stop=(ib == CB - 1))
                v_sb = vpool.tile([P, H, D + 1], bf16, tag="v",
                                  name=f"v_{i}_{idx}")
                nc.vector.tensor_copy(
                    out=v_sb[0:rows, :, 0:D],
                    in_=ps[0:rows, 0:C].rearrange("p (h d) -> p h d", d=D))
                nc.vector.memset(v_sb[0:rows, :, D], 1.0)
                st[("v", i, b, tb)] = v_sb

            NM = (TG + P - 1) // P  # out-projection M-tiles per group (7)

            def emit_out_chunk(i, m):
                """One M-tile of the group-packed output projection: the four
                batches' t-dims concatenate to 788 rows = 7 tiles of <=128
                (vs 8 when tiled per batch: 197 -> 128+69 wastes M)."""
                gd = seq[i]
                rows = min(P, TG - m * P)
                ps = bigps.tile([P, TG], f32, tag="ps", name=f"ops_{i}_{m}")
                for jb in range(CB):
                    ct = st[("ctxT", i, jb // 2)]
                    lhs = ct[:, jb % 2, m * P:m * P + rows]
                    nc.tensor.matmul(
                        ps[0:rows, 0:512], lhsT=lhs,
                        rhs=wo_sb[:, jb, 0:512],
                        start=(jb == 0), stop=(jb == CB - 1))
                    nc.tensor.matmul(
                        ps[0:rows, 512:C], lhsT=lhs,
                        rhs=wo_sb[:, jb, 512:C],
                        start=(jb == 0), stop=(jb == CB - 1))
                out_sb = outsbpool.tile([P, C], f32, tag="out",
                                        name=f"osb_{i}_{m}")
                nc.scalar.activation(out_sb[0:rows, :], ps[0:rows, 0:C],
                                     Copy)
                nc.sync.dma_start(
                    out=out.rearrange("b t c -> (b t) c")[
                        gd * TG + m * P:gd * TG + m * P + rows, :],
                    in_=out_sb[0:rows, :])
                if m == NM - 1:
                    # group fully projected: release the ctxT references so
                    # the epilogue doesn't re-emit (and so tile lifetimes end)
                    for jp in range(CB // 2):
                        st.pop(("ctxT", i, jp), None)

            def emit_out_group(i):
                for m in range(NM):
                    emit_out_chunk(i, m)

            def proj_feeder(i):
                """Thunks interleaved into group i-1's attention so the PE
                duty cycle stays high (keeps the HAM clock-gate at 8/8):
                group i-2's output projection, then group i's projections.
                proj_feeder(1) first yields the tail of group 0's own
                projections (the prologue emits only what batch 0 needs)."""
                if i == 1:
                    for idx in range(2, 12):
                        yield lambda idx=idx: emit_qk_chunk(0, idx)
                    for idx in range(2, 8):
                        yield lambda idx=idx: emit_v_chunk(0, idx)
                if i - 2 >= 0:
                    for m in range(NM):
                        yield lambda m=m: emit_out_chunk(i - 2, m)
                if 1 <= i < nseq:
                    for idx in range(12):
                        yield lambda idx=idx: emit_qk_chunk(i, idx)
                    for idx in range(8):
                        yield lambda idx=idx: emit_v_chunk(i, idx)

            def feed(feeder):
                thunk = next(feeder, None)
                if thunk is not None:
                    thunk()
                    return True
                return False

            def attn_batch(i, b, feeder):
                gd = seq[i]
                # force-drain the feeder until this batch's inputs exist
                while (("v", i, b, 0) not in st or ("v", i, b, 1) not in st
                       or ("qT", i) not in st):
                    if not feed(feeder):
                        break
                qT_sb, kT_sb = st[("qT", i)], st[("kT", i)]
                vb0 = st[("v", i, b, 0)]
                vb1 = st[("v", i, b, 1)]
                ctxq0 = cqpool.tile([P, C], bf16, tag="cq0",
                                    name=f"cq0_{i}_{b}")
                ctxq1 = cqpool.tile([T1, C], bf16, tag="cq1",
                                    name=f"cq1_{i}_{b}")
                ptiles = {}
                chunk = {}

                def s_head(h):
                    j, hi = h // 2, h % 2
                    base = hi * D
                    qh = qT_sb[base:base + D, j, b * T:(b + 1) * T]
                    kh = kT_sb[base:base + D, j, b * T:(b + 1) * T]
                    s = sps.tile([P, SW], f32, tag="s", name=f"s_{i}_{b}_{h}")
                    nc.tensor.matmul(s[:, 0:T], lhsT=kh[:, 0:P], rhs=qh,
                                     start=True, stop=True,
                                     skip_group_check=True)
                    nc.tensor.matmul(s[0:T1, SB1:SB1 + T1],
                                     lhsT=kh[:, P:T], rhs=qh[:, P:T],
                                     start=True, stop=True,
                                     skip_group_check=True)
                    p = ppool.tile([P, SW], bf16, tag="p",
                                   name=f"p_{i}_{b}_{h}")
                    nc.scalar.activation(p[:], s[:], Exp)
                    # causal mask: multiplicative 0/1 on the triangles
                    nc.vector.tensor_mul(p[:, 0:P], p[:, 0:P],
                                         mask_sb[:, 0:P])
                    nc.vector.tensor_mul(p[0:T1, SB1:SB1 + T1],
                                         p[0:T1, SB1:SB1 + T1],
                                         mask_sb[0:T1, 0:T1])
                    ptiles[h] = p

                def ctx_head(h):
                    r = h % HC
                    if r == 0:
                        chunk["ctx"] = cxps.tile([P, HC * CW], f32, tag="cx",
                                                 name=f"cx_{i}_{b}_{h}")
                        chunk["inv"] = invpool.tile([P, HC, 2], f32,
                                                    tag="inv",
                                                    name=f"inv_{i}_{b}_{h}")
                    ctx_ps, inv = chunk["ctx"], chunk["inv"]
                    p = ptiles.pop(h)
                    col = r * CW
                    # q-block0: all needed k are in k-block0
                    nc.tensor.matmul(ctx_ps[:, col:col + D + 1],
                                     lhsT=p[:, 0:P], rhs=vb0[:, h, :],
                                     start=True, stop=True,
                                     skip_group_check=True)
                    # q-block1: full k-block0 + the block1 triangle
                    nc.tensor.matmul(ctx_ps[0:T1, col + D + 1:col + CW],
                                     lhsT=p[:, P:T], rhs=vb0[:, h, :],
                                     start=True, stop=False,
                                     skip_group_check=True)
                    nc.tensor.matmul(ctx_ps[0:T1, col + D + 1:col + CW],
                                     lhsT=p[0:T1, SB1:SB1 + T1],
                                     rhs=vb1[0:T1, h, :],
                                     start=False, stop=True,
                                     skip_group_check=True)
                    if r == HC - 1:
                        h0 = h - (HC - 1)
                        ctx_r = ctx_ps.rearrange("p (h c) -> p h c", c=CW)
                        # denominators sit at columns D / CW-1 per head
                        nc.vector.reciprocal(
                            inv[:, :, 0:1], ctx_r[:, :, D:D + 1])
                        nc.vector.reciprocal(
                            inv[0:T1, :, 1:2], ctx_r[0:T1, :, CW - 1:CW])
                        # normalize + evict to ctxq tiles (bf16)
                        nc.vector.tensor_mul(
                            ctxq0[:, h0 * D:(h0 + HC) * D].rearrange(
                                "p (h d) -> p h d", d=D),
                            ctx_r[:, :, 0:D],
                            inv[:, :, 0:1].to_broadcast((P, HC, D)))
                        nc.vector.tensor_mul(
                            ctxq1[:, h0 * D:(h0 + HC) * D].rearrange(
                                "p (h d) -> p h d", d=D),
                            ctx_r[0:T1, :, D + 1:CW - 1],
                            inv[0:T1, :, 1:2].to_broadcast((T1, HC, D)))

                def transpose_pair(jp):
                    # ctx^T[j,t] for j-blocks 2jp/2jp+1 (heads 4jp..4jp+3);
                    # sits between dense matmuls so the HAM clock-gate stays
                    # open (transpose-mode MMs don't count as PE activity)
                    tps = cxps.tile([P, 2 * TPW], bf16, tag="cx",
                                    name=f"tps_{i}_{b}_{jp}")
                    for u in range(2):
                        jb = 2 * jp + u
                        coff = u * TPW
                        nc.tensor.matmul(
                            tps[:, coff:coff + P],
                            lhsT=ctxq0[:, jb * P:(jb + 1) * P],
                            rhs=ident_sb[:], is_transpose=True,
                            start=True, stop=True, skip_group_check=True)
                        nc.tensor.matmul(
                            tps[:, coff + P:coff + T],
                            lhsT=ctxq1[:, jb * P:(jb + 1) * P],
                            rhs=ident_sb[0:T1, 0:T1], is_transpose=True,
                            start=True, stop=True, skip_group_check=True)
                    if b == 0:
                        st[("ctxT", i, jp)] = ctxTpool.tile(
                            [P, 2, TG], bf16, tag="ctxT",
                            name=f"ctxT_{i}_{jp}")
                    ct = st[("ctxT", i, jp)]
                    nc.vector.tensor_copy(
                        out=ct[:, :, b * T:(b + 1) * T],
                        in_=tps.rearrange("p (u t) -> p u t", t=TPW)[:, :, 0:T])

                # interleave: S(h) then ctx(h-1); feed a projection chunk of
                # the NEXT group after every other head to keep the PE dense;
                # transpose each j-block pair as soon as its 4 heads are done
                def need_qk(h):
                    # S(h) reads q/k c-block h//2: ensure those projection
                    # chunks are EMITTED before s_head(h) (group 0 self-feeds)
                    while st.get(("qkn", i), 0) < 2 * (h // 2) + 2:
                        if not feed(feeder):
                            break

                need_qk(0)
                s_head(0)
                for h in range(1, H):
                    need_qk(h)
                    s_head(h)
                    ctx_head(h - 1)
                    feed(feeder)
                    if h in (5, 9):
                        transpose_pair((h - 5) // 4)
                ctx_head(H - 1)
                transpose_pair(2)


            # --- schedule ---
            # weight DMAs ordered so the first projections' deps land first;
            # wq/xT(0) split per-ib since the first matmuls consume them in
            # ib order (walrus subtile deps let compute start after chunk 0)
            for ib in range(CB):
                nc.sync.dma_start(out=wq_sb[:, ib, :], in_=wq[ib])
            nc.sync.dma_start(out=mask_sb[:], in_=maskp[:])
            nc.sync.dma_start(out=ident_sb[:], in_=identp[:])
            emit_xT(0, split=True)
            for ib in range(CB):
                nc.sync.dma_start(out=wk_sb[:, ib, :], in_=wk[ib])
            for ib in range(CB):
                nc.sync.dma_start(out=wv_sb[:, ib, :], in_=wv[ib])
            for ib in range(CB):
                nc.sync.dma_start(out=wo_sb[:, ib, :], in_=wo[ib])
            if nseq > 1:
                emit_xT(1)
            # minimal prologue: q/k c-block 0 and batch 0's v tiles — just
            # what attention(0, b=0) needs to start; the rest is fed inline
            emit_qk_chunk(0, 0)
            emit_qk_chunk(0, 1)
            emit_v_chunk(0, 0)
            emit_v_chunk(0, 1)
            for i in range(nseq):
                if i + 2 < nseq:
                    emit_xT(i + 2)
                feeder = proj_feeder(i + 1)
                for b in range(G):
                    attn_batch(i, b, feeder)
                for thunk in feeder:  # drain any unfed chunks
                    thunk()
                # drop references for this pipeline position
                for key in [("x", i), ("qT", i), ("kT", i)] + [
                        ("v", i, b, tb) for b in range(G) for tb in range(2)]:
                    st.pop(key, None)
            # epilogue: output projections not yet drained by feeders
            for i in range(nseq):
                if ("ctxT", i, 0) in st:
                    emit_out_group(i)

    _dedup_ldweights(nc)
    if split_waits:
        _split_ctrl_waits(nc)
    return nc


def _prep_core_inputs(hidden_states, Wq, Wk, Wv, Wo):
    """Host-side layout prep. Returns per-core in_maps (list of dicts)."""
    import ml_dtypes

    bf16 = ml_dtypes.bfloat16
    scale = 1.0 / np.sqrt(D)
    # xT[ib, p, b*T+t] = x[b, t, ib*128+p]
    x = np.ascontiguousarray(hidden_states.astype(np.float32))
    wq_h = np.ascontiguousarray((Wq * scale).reshape(CB, P, C).astype(bf16))
    wk_h = np.ascontiguousarray(Wk.reshape(CB, P, C).astype(bf16))
    wv_h = np.ascontiguousarray(Wv.reshape(CB, P, C).astype(bf16))
    wo_h = np.ascontiguousarray(Wo.reshape(CB, P, C).astype(bf16))
    mask = (np.arange(T)[None, :] >= np.arange(P)[:, None]).astype(bf16)
    ident = np.eye(P, dtype=bf16)

    in_maps = []
    for c in range(NCORES):
        xs = x[c * NB:(c + 1) * NB]  # [NB, T, C]
        xT = xs.reshape(NB, T, CB, P).transpose(2, 3, 0, 1).reshape(CB, P, NB * T)
        in_maps.append({
            "xT": np.ascontiguousarray(xT.astype(bf16)),
            "wq": wq_h, "wk": wk_h, "wv": wv_h, "wo": wo_h,
            "mask": mask, "ident": ident,
        })
    return in_maps


def kernel(hidden_states, Wq, bq, Wk, bk, Wv, bv, Wo, bo, counter, ucb,
           **extra):
    hidden_states = np.asarray(hidden_states)
    Wq, bq = np.asarray(Wq), np.asarray(bq)
    Wk, bk = np.asarray(Wk), np.asarray(bk)
    Wv, bv = np.asarray(Wv), np.asarray(bv)
    Wo, bo = np.asarray(Wo), np.asarray(bo)

    if np.any(bq) or np.any(bk):
        # exact numpy fallback (not expected to trigger: spec fills zeros)
        return _numpy_reference(hidden_states, Wq, bq, Wk, bk, Wv, bv, Wo, bo)

    if "nc" not in _CACHE:
        _CACHE["nc"] = build_nc()
    nc = _CACHE["nc"]
    if "runner" not in _CACHE:
        _CACHE["runner"] = _make_runner(nc)
    run, out_names, out_avals = _CACHE["runner"]

    in_maps = _prep_core_inputs(hidden_states, Wq, Wk, Wv, Wo)
    out_arrs, _ = run(in_maps)
    full = np.asarray(out_arrs[out_names.index("out")])
    out = full  # [NCORES*NB, T, C] — concat over cores is exactly batch order

    # bv/bo enter the output linearly: out += bv @ Wo + bo (attention rows sum
    # to one, so the bv term is constant across positions).
    if np.any(bv) or np.any(bo):
        out = out + (bv.astype(np.float64) @ Wo.astype(np.float64)
                     + bo.astype(np.float64)).astype(np.float32)[None, None, :]
    return out.astype(np.float32)


def _make_runner(nc):
    """Cached jitted runner (mirrors bass2jax.run_bass_via_pjrt) that keeps
    inputs device-resident so repeated calls time pure device execution."""
    import jax
    import concourse.mybir as mybir
    from concourse import bass2jax
    from concourse.bass2jax import _bass_exec_p, install_neuronx_cc_hook
    from jax.sharding import Mesh, PartitionSpec
    from jax.experimental.shard_map import shard_map

    install_neuronx_cc_hook()
    n_cores = NCORES
    partition_name = (nc.partition_id_tensor.name
                      if nc.partition_id_tensor else None)
    in_names, out_names, out_avals = [], [], []
    for alloc in nc.m.functions[0].allocations:
        if not isinstance(alloc, mybir.MemoryLocationSet):
            continue
        name = alloc.memorylocations[0].name
        if alloc.kind == "ExternalInput":
            if name != partition_name:
                in_names.append(name)
        elif alloc.kind == "ExternalOutput":
            shape = tuple(alloc.tensor_shape)
            dtype = mybir.dt.np(alloc.dtype)
            out_names.append(name)
            out_avals.append(jax.core.ShapedArray(shape, dtype))
    n_params = len(in_names)
    all_names = in_names + out_names
    if partition_name is not None:
        all_names = all_names + [partition_name]

    def _body(*args):
        operands = list(args)
        if partition_name is not None:
            operands.append(bass2jax.partition_id_tensor())
        outs = _bass_exec_p.bind(
            *operands,
            out_avals=tuple(out_avals),
            in_names=tuple(all_names),
            out_names=tuple(out_names),
            lowering_input_output_aliases=(),
            sim_require_finite=False,
            sim_require_nnan=False,
            nc=nc,
        )
        return tuple(outs)

    devices = jax.devices()[:n_cores]
    mesh = Mesh(np.asarray(devices), ("core",))
    in_specs = (PartitionSpec("core"),) * (n_params + len(out_names))
    out_specs = (PartitionSpec("core"),) * len(out_names)
    sharded = jax.jit(
        shard_map(_body, mesh=mesh, in_specs=in_specs, out_specs=out_specs,
                  check_rep=False),
        keep_unused=True,
    )

    def make_repeat(repeat):
        n_outs = len(out_names)

        def _body_r(*args):
            params = list(args[:n_params])
            outbufs = list(args[n_params:])
            outs = None
            for _ in range(repeat):
                # thread the previous iteration's outputs in as the output
                # operands: forces a data dependency so XLA cannot dedupe
                # or reorder the repeated effectful calls
                outs = _body(*params, *outbufs)
                outbufs = list(outs)
            return outs
        return jax.jit(
            shard_map(_body_r, mesh=mesh, in_specs=in_specs,
                      out_specs=out_specs, check_rep=False),
            keep_unused=True,
        )

    def run(in_maps, device_inputs=None):
        if device_inputs is None:
            concat_in = [
                np.concatenate([np.asarray(in_maps[c][nm]) for c in range(n_cores)],
                               axis=0)
                for nm in in_names
            ]
            concat_zeros = [
                np.zeros((n_cores * a.shape[0], *a.shape[1:]), a.dtype)
                for a in out_avals
            ]
            device_inputs = jax.device_put(
                concat_in + concat_zeros,
                [jax.sharding.NamedSharding(mesh, PartitionSpec("core"))]
                * (n_params + len(out_names)),
            )
        out_arrs = sharded(*device_inputs)
        jax.block_until_ready(out_arrs)
        return out_arrs, device_inputs

    run.make_repeat = make_repeat
    return run, out_names, out_avals


def _numpy_reference(hidden_states, Wq, bq, Wk, bk, Wv, bv, Wo, bo):
    x = hidden_states.astype(np.float64)
    q = (x @ Wq.astype(np.float64) + bq).reshape(B, T, H, D).transpose(0, 2, 1, 3)
    k = (x @ Wk.astype(np.float64) + bk).reshape(B, T, H, D).transpose(0, 2, 1, 3)
    v = (x @ Wv.astype(np.float64) + bv).reshape(B, T, H, D).transpose(0, 2, 1, 3)
    s = np.einsum("bhqd,bhkd->bhqk", q, k) / np.sqrt(D)
    causal = np.tril(np.ones((T, T), dtype=bool))
    s = np.where(causal, s, -np.inf)
    s = s - s.max(axis=-1, keepdims=True)
    p = np.exp(s)
    p = p / p.sum(axis=-1, keepdims=True)
    ctx = np.einsum("bhqk,bhkd->bhqd", p, v).transpose(0, 2, 1, 3).reshape(B, T, C)
    return (ctx @ Wo.astype(np.float64) + bo).astype(np.float32)
